# revision 58
# baseline (speedup 1.0000x reference)
"""PointConv (gnn_message_passing) Bass kernel for 8 TRN2 NeuronCores.

Math (per reference, with deg == K == 32 exactly for the standard edge list):
  pos_local = pos_in[in_index] - pos_in[out_index]            [E, 3]
  xj = x_in[in_index, 0] / 32                                 [E]
  M = celu(celu(pos_local @ W1) @ W2)                         [E, 64]
  P = segment_sum(xj[:, None] * M, out_index)                 [N, 64]
  out = P @ W3 + b3                                           [N, 64]

Device computes the shifted form (celu(x)+1 = relu(x) + min(e^x, 1)):
  c1' = celu1 + 1   (per-edge hidden, 16-wide)
  psB = c1' @ W2 = z + colsum(W2)   -> ACT bias -colsum(W2) recovers z
  c2' = celu2(z) + 1 = relu(z) + min(e^z, 1)
  pt  = segment_sum(xj * c2') = P + S_x      (S_x[n] = sum_k xj[nK+k])
  dev_out = pt @ W3 + b3 = out + S_x (x) colsum-rows(W3)
Host subtracts the rank-1 S_x (x) w3sum term (exact).

Engine split per super-chunk (4096 edges):
  ACT : exp (PSUM->SBUF fp16, bias), celu1 exp+relu, A-variant relu2
  DVE : min(e,1) [4x fp16], fused (min(e2,1)+r2) via scalar_tensor_tensor,
        B-variant relu from PSUM via tensor_scalar(sub, max), xm multiply
  Pool: segmented 32:1 sum reduce (SBUF fp16)
  PE  : mm1 (block-diag 8x W1), mm2 (single 2x W2 stationary, moving at
        partition offset 32j), mm3 at the end
  DMA : xj partition-broadcast in fp16
"""

import numpy as np

N = 50000
K = 32
E = N * K
NCORES = 8
N_LOC = N // NCORES          # 6250
E_LOC = E // NCORES          # 200000
SUB = 512
SC = 4096                    # edges per super-chunk
N_SC = 50                    # super-chunks per core (padded, even for pairs)
E_PAD = N_SC * SC            # 204800
N_TILES = E_PAD // 1024      # 200
N_PAD = E_PAD // K           # 6400
OUTC = N_SC * 64             # 3200 packed output cols
# A-variant (relu2 on ACT) per SC-pair, else B (linear path on DVE from PSUM)
A_VARIANT = [(s // 2) % 4 != 3 for s in range(N_SC)]

_CACHE = {}


def _build():
    import concourse.mybir as mybir
    import concourse.tile as tile
    from concourse import bacc

    f32 = mybir.dt.float32
    f16 = mybir.dt.float16
    Alu = mybir.AluOpType
    Act = mybir.ActivationFunctionType
    AxisX = mybir.AxisListType.X

    nc = bacc.Bacc("TRN2", target_bir_lowering=False, debug=False)

    c1t8 = nc.dram_tensor("c1t8", (128, N_SC * SUB), f16, kind="ExternalInput")
    xj2 = nc.dram_tensor("xj2", (2, N_TILES * SUB), f16, kind="ExternalInput")
    w2bd = nc.dram_tensor("w2bd", (128, 256), f16, kind="ExternalInput")
    w2csn = nc.dram_tensor("w2csn", (128, 1), f32, kind="ExternalInput")
    w2csm1 = nc.dram_tensor("w2csm1", (128, 1), f32, kind="ExternalInput")
    w3d = nc.dram_tensor("w3d", (128, 64), f16, kind="ExternalInput")
    b3d = nc.dram_tensor("b3d", (64, 1), f32, kind="ExternalInput")
    outT = nc.dram_tensor("outT", (128, OUTC), f16, kind="ExternalOutput")

    with tile.TileContext(nc) as tc:
        with (
            tc.tile_pool(name="const", bufs=1) as cpool,
            tc.tile_pool(name="data", bufs=1) as dpool,
            tc.tile_pool(name="pb", bufs=4, space="PSUM") as pb_pool,
            tc.tile_pool(name="e2p", bufs=2) as e2p,
            tc.tile_pool(name="r2p", bufs=2) as r2p,
            tc.tile_pool(name="c2p", bufs=2) as c2p,
            tc.tile_pool(name="xmp", bufs=2) as xmp,
            tc.tile_pool(name="trp", bufs=1) as trp,
            tc.tile_pool(name="sxp", bufs=2) as sxp,
        ):
            w2_sb = cpool.tile([128, 256], f16)
            nc.sync.dma_start(out=w2_sb[:], in_=w2bd[:])
            w3_sb = cpool.tile([128, 64], f16)
            nc.sync.dma_start(out=w3_sb[:], in_=w3d[:])
            w2csn_sb = cpool.tile([128, 1], f32)
            nc.sync.dma_start(out=w2csn_sb[:], in_=w2csn[:])
            w2csm1_sb = cpool.tile([128, 1], f32)
            nc.sync.dma_start(out=w2csm1_sb[:], in_=w2csm1[:])
            b3_sb = cpool.tile([64, 1], f32)
            nc.sync.dma_start(out=b3_sb[:], in_=b3d[:])

            c1_sb = dpool.tile([128, N_SC * SUB], f16)
            nc.sync.dma_start(out=c1_sb[:], in_=c1t8[:])
            pt = dpool.tile([128, OUTC], f16)
            out_sb = dpool.tile([128, OUTC], f16)

            for p in range(N_SC // 2):
                # --- all element-wise tiles are per SC-PAIR [128, 4096]
                va = A_VARIANT[2 * p]
                sx = sxp.tile([128, SC], f16)
                lo, hi = p * SC, (p + 1) * SC
                nc.sync.dma_start(
                    out=sx[0:64, :],
                    in_=xj2[0, lo:hi].partition_broadcast(64))
                nc.sync.dma_start(
                    out=sx[64:128, :],
                    in_=xj2[1, lo:hi].partition_broadcast(64))

                e2 = e2p.tile([128, 4096], f16)
                r2 = r2p.tile([128, 4096], f16, name="r2")
                for q in range(2):
                    s = 2 * p + q
                    for half in range(2):
                        psB = pb_pool.tile([128, 1024], f32, name="psB")
                        for jj in range(2):
                            j = 2 * half + jj
                            g, v = j // 2, j % 2
                            nc.tensor.matmul(
                                psB[:, 512 * jj:512 * (jj + 1)],
                                w2_sb[64 * g:64 * (g + 1),
                                      128 * v:128 * (v + 1)],
                                c1_sb[64 * g:64 * (g + 1),
                                      s * SUB:(s + 1) * SUB],
                                start=True, stop=True,
                            )
                        cols = slice(2048 * q + 1024 * half,
                                     2048 * q + 1024 * (half + 1))
                        nc.scalar.activation(
                            e2[:, cols], psB[:], Act.Exp, bias=w2csn_sb[:])
                        if va:
                            nc.scalar.activation(
                                r2[:, cols], psB[:], Act.Relu,
                                bias=w2csn_sb[:])
                        else:
                            # r2 = max(z + 1, 1)  (linear path off PSUM)
                            nc.vector.tensor_scalar(
                                out=r2[:, cols], in0=psB[:],
                                scalar1=w2csm1_sb[:], scalar2=1.0,
                                op0=Alu.subtract, op1=Alu.max,
                            )
                c2 = c2p.tile([128, 4096], f16)
                if va:
                    # c2' = min(e2, 1) + r2
                    m2 = trp.tile([128, 4096], f16, name="m2")
                    nc.vector.tensor_scalar_min(
                        out=m2[:], in0=e2[:], scalar1=1.0)
                    nc.vector.tensor_add(out=c2[:], in0=r2[:], in1=m2[:])
                else:
                    # c2' = min(e2, max(z+1, 1))  (exact identity)
                    nc.vector.tensor_tensor(
                        out=c2[:], in0=e2[:], in1=r2[:], op=Alu.min)
                xm = xmp.tile([128, 4096], f16)
                nc.vector.tensor_mul(out=xm[:], in0=c2[:], in1=sx[:])
                # tree level 1 (k-major -> contiguous halves per 512-block)
                tr256 = trp.tile([128, 2048], f16, name="tr256")
                seg = xm[:].rearrange("p (g k) -> p g k", k=512)
                dstv = tr256[:].rearrange("p (g k) -> p g k", k=256)
                nc.vector.tensor_add(
                    out=dstv, in0=seg[:, :, 0:256], in1=seg[:, :, 256:512])
                # tree levels 2-5, consolidated across the SC pair
                tsum = tr256
                for width in (128, 64, 32, 16):
                    seg = tsum[:].rearrange("p (g k) -> p g k", k=2 * width)
                    if width > 16:
                        nxt = trp.tile([128, 8 * width], f16,
                                       name=f"tr{width}")
                        dst = nxt[:].rearrange("p (g k) -> p g k", k=width)
                    else:
                        nxt = None
                        dst = (pt[:, 2 * p * 64:2 * p * 64 + 128]
                               .rearrange("p (g k) -> p g k", k=width))
                    nc.vector.tensor_add(
                        out=dst, in0=seg[:, :, 0:width],
                        in1=seg[:, :, width:2 * width],
                    )
                    tsum = nxt

            # --- mm3: out = pt @ W3 + b3; 512-col matmuls, [1024] ACT per h
            n_cb = OUTC // 1024 + (1 if OUTC % 1024 else 0)
            for cb in range(n_cb):
                w = min(1024, OUTC - cb * 1024)
                for h in range(2):
                    ps3t = pb_pool.tile([128, 1024], f32, name="psB")
                    ps3 = ps3t[64 * h:64 * (h + 1), :]
                    for hw in range(0, w, 512):
                        ww = min(512, w - hw)
                        nc.tensor.matmul(
                            ps3[:, hw:hw + ww], w3_sb[64 * h:64 * (h + 1), :],
                            pt[64 * h:64 * (h + 1),
                               cb * 1024 + hw:cb * 1024 + hw + ww],
                            start=True, stop=True,
                        )
                    nc.scalar.activation(
                        out=out_sb[64 * h:64 * (h + 1),
                                   cb * 1024:cb * 1024 + w],
                        in_=ps3[:, 0:w], func=Act.Identity, bias=b3_sb[:],
                    )
            nc.sync.dma_start(out=outT[:], in_=out_sb[:])

    nc.compile()
    return nc


def _reference_numpy(x_in, pos_in, W1, W2, W3, b3, in_index, out_index):
    def celu(x):
        return np.maximum(x, 0.0) + np.minimum(np.expm1(np.minimum(x, 0.0)), 0.0)

    n = pos_in.shape[0]
    pos_local = np.nan_to_num(pos_in[in_index] - pos_in[out_index])
    deg = np.bincount(out_index, minlength=n).astype(np.float32)
    deg = np.maximum(deg, 1.0)
    xj = x_in[in_index, 0] * (1.0 / deg)[out_index]
    M = celu(celu(pos_local @ W1) @ W2)
    prod = xj[:, None] * M
    P = np.zeros((n, M.shape[1]), dtype=np.float32)
    np.add.at(P, out_index, prod)
    out = P @ W3 + b3
    return np.nan_to_num(out, posinf=10000.0, neginf=-10000.0).astype(np.float32)


def build_in_maps(inputs):
    x_in = np.asarray(inputs["x_in"], dtype=np.float32)
    pos_in = np.asarray(inputs["pos_in"], dtype=np.float32)
    W1 = np.asarray(inputs["W1"], dtype=np.float32)
    W2 = np.asarray(inputs["W2"], dtype=np.float32)
    W3 = np.asarray(inputs["W3"], dtype=np.float32)
    b3 = np.asarray(inputs["b3"], dtype=np.float32)
    in_index = np.asarray(inputs["in_index"])
    out_index = np.asarray(inputs["out_index"])

    pos_local = (pos_in[in_index] - pos_in[out_index]).astype(np.float32)
    xj = (x_in[in_index, 0] * (1.0 / K)).astype(np.float16)
    # host-side 16-wide stage: c1' = celu(pos_local @ W1) + 1
    z1 = pos_local @ W1
    c1p_full = (np.maximum(z1, 0.0)
                + np.exp(np.minimum(z1, 0.0), dtype=np.float32)
                ).astype(np.float16)
    del z1

    # stationary variants: mm2 for quarter j = 2g+v uses rows 64g:64g+64,
    # cols 128v:128v+128; active contraction rows are 32v:32v+32 of the group
    w2bd = np.zeros((128, 256), np.float16)
    for g in range(2):
        for v in range(2):
            for a in range(2):
                w2bd[64 * g + 32 * v + 16 * a:64 * g + 32 * v + 16 * (a + 1),
                     128 * v + 64 * a:128 * v + 64 * (a + 1)] = W2
    w2cs = np.tile(W2.sum(axis=0).astype(np.float32).reshape(64, 1), (2, 1))
    w3d = np.tile(W3.astype(np.float16), (2, 1))
    b3d = np.asarray(b3, np.float32).reshape(64, 1)

    in_maps = []
    for d in range(NCORES):
        c1_d = np.ones((E_PAD, 16), np.float16)
        c1_d[:E_LOC] = c1p_full[d * E_LOC:(d + 1) * E_LOC]
        xj_d = np.zeros((E_PAD,), np.float16)
        xj_d[:E_LOC] = xj[d * E_LOC:(d + 1) * E_LOC]
        # k-major order within each 512-edge block: col = k*16 + n_local
        c1_d = (c1_d.reshape(-1, 16, K, 16).transpose(0, 2, 1, 3)
                .reshape(E_PAD, 16))
        xj_d = xj_d.reshape(-1, 16, K).transpose(0, 2, 1).reshape(E_PAD)
        c1t8 = np.ascontiguousarray(
            c1_d.reshape(N_SC, 8, SUB, 16).transpose(1, 3, 0, 2)
            .reshape(128, N_SC * SUB))
        xj2 = np.ascontiguousarray(
            xj_d.reshape(N_TILES, 2, SUB).transpose(1, 0, 2)
            .reshape(2, N_TILES * SUB))
        in_maps.append({
            "c1t8": c1t8, "xj2": xj2, "w2bd": w2bd,
            "w2csn": -w2cs, "w2csm1": w2cs - 1.0, "w3d": w3d, "b3d": b3d,
        })
    return in_maps


def kernel(**inputs):
    x_in = np.asarray(inputs["x_in"], dtype=np.float32)
    pos_in = np.asarray(inputs["pos_in"], dtype=np.float32)
    W1 = np.asarray(inputs["W1"], dtype=np.float32)
    W2 = np.asarray(inputs["W2"], dtype=np.float32)
    W3 = np.asarray(inputs["W3"], dtype=np.float32)
    b3 = np.asarray(inputs["b3"], dtype=np.float32)
    in_index = np.asarray(inputs["in_index"])
    out_index = np.asarray(inputs["out_index"])

    expected = np.repeat(np.arange(N, dtype=np.int64), K).astype(out_index.dtype)
    if x_in.shape != (N, 1) or not np.array_equal(out_index, expected):
        return _reference_numpy(x_in, pos_in, W1, W2, W3, b3,
                                in_index, out_index)

    in_maps = build_in_maps(inputs)

    if "nc" not in _CACHE:
        _CACHE["nc"] = _build()
    from concourse.bass_utils import run_bass_kernel_spmd
    res = run_bass_kernel_spmd(_CACHE["nc"], in_maps, list(range(NCORES)))

    # host-side rank-1 correction: dev_out = out + S_x (x) w3sum
    S_x = (x_in[in_index, 0].astype(np.float64).reshape(N, K).sum(axis=1)
           / K).astype(np.float32)
    w3sum = W3.sum(axis=0).astype(np.float32)

    out = np.empty((N, 64), np.float32)
    for d in range(NCORES):
        oT = res.results[d]["outT"].astype(np.float32)  # [128, 3200]
        # col s*64 + j*16 + n_l, partition 64h+f <- node s*128 + (2j+h)*16 + n_l
        full = (oT.reshape(2, 64, N_SC, 4, 16).transpose(2, 3, 0, 4, 1)
                .reshape(N_PAD, 64))
        out[d * N_LOC:(d + 1) * N_LOC] = full[:N_LOC]
    out -= S_x[:, None] * w3sum[None, :]
    return np.nan_to_num(out, posinf=10000.0, neginf=-10000.0)


# revision 61
# speedup vs baseline: 1.0313x; 1.0313x over previous
"""PointConv (gnn_message_passing) Bass kernel for 8 TRN2 NeuronCores.

Math (per reference, with deg == K == 32 exactly for the standard edge list):
  pos_local = pos_in[in_index] - pos_in[out_index]            [E, 3]
  xj = x_in[in_index, 0] / 32                                 [E]
  M = celu(celu(pos_local @ W1) @ W2)                         [E, 64]
  P = segment_sum(xj[:, None] * M, out_index)                 [N, 64]
  out = P @ W3 + b3                                           [N, 64]

Device computes the shifted form (celu(x)+1 = relu(x) + min(e^x, 1)):
  c1' = celu1 + 1   (per-edge hidden, 16-wide)
  psB = c1' @ W2 = z + colsum(W2)   -> ACT bias -colsum(W2) recovers z
  c2' = celu2(z) + 1 = relu(z) + min(e^z, 1)
  pt  = segment_sum(xj * c2') = P + S_x      (S_x[n] = sum_k xj[nK+k])
  dev_out = pt @ W3 + b3 = out + S_x (x) colsum-rows(W3)
Host subtracts the rank-1 S_x (x) w3sum term (exact).

Engine split per super-chunk (4096 edges):
  ACT : exp (PSUM->SBUF fp16, bias), celu1 exp+relu, A-variant relu2
  DVE : min(e,1) [4x fp16], fused (min(e2,1)+r2) via scalar_tensor_tensor,
        B-variant relu from PSUM via tensor_scalar(sub, max), xm multiply
  Pool: segmented 32:1 sum reduce (SBUF fp16)
  PE  : mm1 (block-diag 8x W1), mm2 (single 2x W2 stationary, moving at
        partition offset 32j), mm3 at the end
  DMA : xj partition-broadcast in fp16
"""

import numpy as np

N = 50000
K = 32
E = N * K
NCORES = 8
N_LOC = N // NCORES          # 6250
E_LOC = E // NCORES          # 200000
SUB = 512
SC = 4096                    # edges per super-chunk
N_SC = 50                    # super-chunks per core (padded, even for pairs)
E_PAD = N_SC * SC            # 204800
N_TILES = E_PAD // 1024      # 200
N_PAD = E_PAD // K           # 6400
OUTC = N_SC * 64             # 3200 packed output cols
# A-variant (relu2 on ACT) for most SCs, else B (linear path on DVE from PSUM)
A_VARIANT = [s % 10 != 9 for s in range(N_SC)]

_CACHE = {}


def _build():
    import concourse.mybir as mybir
    import concourse.tile as tile
    from concourse import bacc

    f32 = mybir.dt.float32
    f16 = mybir.dt.float16
    Alu = mybir.AluOpType
    Act = mybir.ActivationFunctionType
    AxisX = mybir.AxisListType.X

    nc = bacc.Bacc("TRN2", target_bir_lowering=False, debug=False)

    c1t8 = nc.dram_tensor("c1t8", (128, N_SC * SUB), f16, kind="ExternalInput")
    xj2 = nc.dram_tensor("xj2", (2, N_TILES * SUB), f16, kind="ExternalInput")
    w2bd = nc.dram_tensor("w2bd", (128, 256), f16, kind="ExternalInput")
    w2csn = nc.dram_tensor("w2csn", (128, 1), f32, kind="ExternalInput")
    w2csm1 = nc.dram_tensor("w2csm1", (128, 1), f32, kind="ExternalInput")
    w3d = nc.dram_tensor("w3d", (128, 64), f16, kind="ExternalInput")
    b3d = nc.dram_tensor("b3d", (64, 1), f32, kind="ExternalInput")
    outT = nc.dram_tensor("outT", (128, OUTC), f16, kind="ExternalOutput")

    with tile.TileContext(nc) as tc:
        with (
            tc.tile_pool(name="const", bufs=1) as cpool,
            tc.tile_pool(name="data", bufs=1) as dpool,
            tc.tile_pool(name="pb", bufs=4, space="PSUM") as pb_pool,
            tc.tile_pool(name="e2p", bufs=3) as e2p,
            tc.tile_pool(name="r2p", bufs=3) as r2p,
            tc.tile_pool(name="c2p", bufs=3) as c2p,
            tc.tile_pool(name="xmp", bufs=3) as xmp,
            tc.tile_pool(name="trp", bufs=2) as trp,
            tc.tile_pool(name="sxp", bufs=3) as sxp,
        ):
            w2_sb = cpool.tile([128, 256], f16)
            nc.sync.dma_start(out=w2_sb[:], in_=w2bd[:])
            w3_sb = cpool.tile([128, 64], f16)
            nc.sync.dma_start(out=w3_sb[:], in_=w3d[:])
            w2csn_sb = cpool.tile([128, 1], f32)
            nc.sync.dma_start(out=w2csn_sb[:], in_=w2csn[:])
            w2csm1_sb = cpool.tile([128, 1], f32)
            nc.sync.dma_start(out=w2csm1_sb[:], in_=w2csm1[:])
            b3_sb = cpool.tile([64, 1], f32)
            nc.sync.dma_start(out=b3_sb[:], in_=b3d[:])

            c1_sb = dpool.tile([128, N_SC * SUB], f16)
            nc.sync.dma_start(out=c1_sb[:], in_=c1t8[:])
            pt = dpool.tile([128, OUTC], f16)
            out_sb = dpool.tile([128, OUTC], f16)

            for p in range(N_SC // 2):
                tr256 = trp.tile([128, 2048], f16, name="tr256")
                for q in range(2):
                    s = 2 * p + q
                    # xj broadcast tile for this SC
                    sx = sxp.tile([128, SC // 2], f16)
                    lo, hi = s * (SC // 2), (s + 1) * (SC // 2)
                    nc.sync.dma_start(
                        out=sx[0:64, :],
                        in_=xj2[0, lo:hi].partition_broadcast(64))
                    nc.sync.dma_start(
                        out=sx[64:128, :],
                        in_=xj2[1, lo:hi].partition_broadcast(64))

                    e2 = e2p.tile([128, 2048], f16)
                    r2 = r2p.tile([128, 2048], f16, name="r2")
                    for half in range(2):
                        psB = pb_pool.tile([128, 1024], f32, name="psB")
                        for jj in range(2):
                            j = 2 * half + jj
                            g, v = j // 2, j % 2
                            nc.tensor.matmul(
                                psB[:, 512 * jj:512 * (jj + 1)],
                                w2_sb[64 * g:64 * (g + 1),
                                      128 * v:128 * (v + 1)],
                                c1_sb[64 * g:64 * (g + 1),
                                      s * SUB:(s + 1) * SUB],
                                start=True, stop=True,
                            )
                        cols = slice(1024 * half, 1024 * (half + 1))
                        nc.scalar.activation(
                            e2[:, cols], psB[:], Act.Exp, bias=w2csn_sb[:])
                        if A_VARIANT[s]:
                            nc.scalar.activation(
                                r2[:, cols], psB[:], Act.Relu,
                                bias=w2csn_sb[:])
                        else:
                            # r2 = max(z + 1, 1)  (linear path off PSUM)
                            nc.vector.tensor_scalar(
                                out=r2[:, cols], in0=psB[:],
                                scalar1=w2csm1_sb[:], scalar2=1.0,
                                op0=Alu.subtract, op1=Alu.max,
                            )
                    c2 = c2p.tile([128, 2048], f16)
                    if A_VARIANT[s]:
                        # c2' = min(e2, 1) + r2
                        m2 = trp.tile([128, 2048], f16, name="m2")
                        nc.vector.tensor_scalar_min(
                            out=m2[:], in0=e2[:], scalar1=1.0)
                        nc.vector.tensor_add(out=c2[:], in0=r2[:], in1=m2[:])
                    else:
                        # c2' = min(e2, max(z+1, 1))  (exact identity)
                        nc.vector.tensor_tensor(
                            out=c2[:], in0=e2[:], in1=r2[:], op=Alu.min)
                    xm = xmp.tile([128, 2048], f16)
                    nc.vector.tensor_mul(out=xm[:], in0=c2[:], in1=sx[:])
                    # tree level 1 (k-major -> contiguous halves per block)
                    seg = xm[:].rearrange("p (g k) -> p g k", k=512)
                    dstv = (tr256[:, 1024 * q:1024 * (q + 1)]
                            .rearrange("p (g k) -> p g k", k=256))
                    nc.vector.tensor_add(
                        out=dstv, in0=seg[:, :, 0:256], in1=seg[:, :, 256:512])
                # tree levels 2-5, consolidated across the SC pair
                tsum = tr256
                for width in (128, 64, 32, 16):
                    seg = tsum[:].rearrange("p (g k) -> p g k", k=2 * width)
                    if width > 16:
                        nxt = trp.tile([128, 8 * width], f16,
                                       name=f"tr{width}")
                        dst = nxt[:].rearrange("p (g k) -> p g k", k=width)
                    else:
                        nxt = None
                        dst = (pt[:, 2 * p * 64:2 * p * 64 + 128]
                               .rearrange("p (g k) -> p g k", k=width))
                    nc.vector.tensor_add(
                        out=dst, in0=seg[:, :, 0:width],
                        in1=seg[:, :, width:2 * width],
                    )
                    tsum = nxt

            # --- mm3: out = pt @ W3 + b3; 512-col matmuls, [1024] ACT per h
            n_cb = OUTC // 1024 + (1 if OUTC % 1024 else 0)
            for cb in range(n_cb):
                w = min(1024, OUTC - cb * 1024)
                for h in range(2):
                    ps3t = pb_pool.tile([128, 1024], f32, name="psB")
                    ps3 = ps3t[64 * h:64 * (h + 1), :]
                    for hw in range(0, w, 512):
                        ww = min(512, w - hw)
                        nc.tensor.matmul(
                            ps3[:, hw:hw + ww], w3_sb[64 * h:64 * (h + 1), :],
                            pt[64 * h:64 * (h + 1),
                               cb * 1024 + hw:cb * 1024 + hw + ww],
                            start=True, stop=True,
                        )
                    nc.scalar.activation(
                        out=out_sb[64 * h:64 * (h + 1),
                                   cb * 1024:cb * 1024 + w],
                        in_=ps3[:, 0:w], func=Act.Identity, bias=b3_sb[:],
                    )
            nc.sync.dma_start(out=outT[:], in_=out_sb[:])

    nc.compile()
    return nc


def _reference_numpy(x_in, pos_in, W1, W2, W3, b3, in_index, out_index):
    def celu(x):
        return np.maximum(x, 0.0) + np.minimum(np.expm1(np.minimum(x, 0.0)), 0.0)

    n = pos_in.shape[0]
    pos_local = np.nan_to_num(pos_in[in_index] - pos_in[out_index])
    deg = np.bincount(out_index, minlength=n).astype(np.float32)
    deg = np.maximum(deg, 1.0)
    xj = x_in[in_index, 0] * (1.0 / deg)[out_index]
    M = celu(celu(pos_local @ W1) @ W2)
    prod = xj[:, None] * M
    P = np.zeros((n, M.shape[1]), dtype=np.float32)
    np.add.at(P, out_index, prod)
    out = P @ W3 + b3
    return np.nan_to_num(out, posinf=10000.0, neginf=-10000.0).astype(np.float32)


def build_in_maps(inputs):
    x_in = np.asarray(inputs["x_in"], dtype=np.float32)
    pos_in = np.asarray(inputs["pos_in"], dtype=np.float32)
    W1 = np.asarray(inputs["W1"], dtype=np.float32)
    W2 = np.asarray(inputs["W2"], dtype=np.float32)
    W3 = np.asarray(inputs["W3"], dtype=np.float32)
    b3 = np.asarray(inputs["b3"], dtype=np.float32)
    in_index = np.asarray(inputs["in_index"])
    out_index = np.asarray(inputs["out_index"])

    pos_local = (pos_in[in_index] - pos_in[out_index]).astype(np.float32)
    xj = (x_in[in_index, 0] * (1.0 / K)).astype(np.float16)
    # host-side 16-wide stage: c1' = celu(pos_local @ W1) + 1
    z1 = pos_local @ W1
    c1p_full = (np.maximum(z1, 0.0)
                + np.exp(np.minimum(z1, 0.0), dtype=np.float32)
                ).astype(np.float16)
    del z1

    # stationary variants: mm2 for quarter j = 2g+v uses rows 64g:64g+64,
    # cols 128v:128v+128; active contraction rows are 32v:32v+32 of the group
    w2bd = np.zeros((128, 256), np.float16)
    for g in range(2):
        for v in range(2):
            for a in range(2):
                w2bd[64 * g + 32 * v + 16 * a:64 * g + 32 * v + 16 * (a + 1),
                     128 * v + 64 * a:128 * v + 64 * (a + 1)] = W2
    w2cs = np.tile(W2.sum(axis=0).astype(np.float32).reshape(64, 1), (2, 1))
    w3d = np.tile(W3.astype(np.float16), (2, 1))
    b3d = np.asarray(b3, np.float32).reshape(64, 1)

    in_maps = []
    for d in range(NCORES):
        c1_d = np.ones((E_PAD, 16), np.float16)
        c1_d[:E_LOC] = c1p_full[d * E_LOC:(d + 1) * E_LOC]
        xj_d = np.zeros((E_PAD,), np.float16)
        xj_d[:E_LOC] = xj[d * E_LOC:(d + 1) * E_LOC]
        # k-major order within each 512-edge block: col = k*16 + n_local
        c1_d = (c1_d.reshape(-1, 16, K, 16).transpose(0, 2, 1, 3)
                .reshape(E_PAD, 16))
        xj_d = xj_d.reshape(-1, 16, K).transpose(0, 2, 1).reshape(E_PAD)
        c1t8 = np.ascontiguousarray(
            c1_d.reshape(N_SC, 8, SUB, 16).transpose(1, 3, 0, 2)
            .reshape(128, N_SC * SUB))
        xj2 = np.ascontiguousarray(
            xj_d.reshape(N_TILES, 2, SUB).transpose(1, 0, 2)
            .reshape(2, N_TILES * SUB))
        in_maps.append({
            "c1t8": c1t8, "xj2": xj2, "w2bd": w2bd,
            "w2csn": -w2cs, "w2csm1": w2cs - 1.0, "w3d": w3d, "b3d": b3d,
        })
    return in_maps


def kernel(**inputs):
    x_in = np.asarray(inputs["x_in"], dtype=np.float32)
    pos_in = np.asarray(inputs["pos_in"], dtype=np.float32)
    W1 = np.asarray(inputs["W1"], dtype=np.float32)
    W2 = np.asarray(inputs["W2"], dtype=np.float32)
    W3 = np.asarray(inputs["W3"], dtype=np.float32)
    b3 = np.asarray(inputs["b3"], dtype=np.float32)
    in_index = np.asarray(inputs["in_index"])
    out_index = np.asarray(inputs["out_index"])

    expected = np.repeat(np.arange(N, dtype=np.int64), K).astype(out_index.dtype)
    if x_in.shape != (N, 1) or not np.array_equal(out_index, expected):
        return _reference_numpy(x_in, pos_in, W1, W2, W3, b3,
                                in_index, out_index)

    in_maps = build_in_maps(inputs)

    if "nc" not in _CACHE:
        _CACHE["nc"] = _build()
    from concourse.bass_utils import run_bass_kernel_spmd
    res = run_bass_kernel_spmd(_CACHE["nc"], in_maps, list(range(NCORES)))

    # host-side rank-1 correction: dev_out = out + S_x (x) w3sum
    S_x = (x_in[in_index, 0].astype(np.float64).reshape(N, K).sum(axis=1)
           / K).astype(np.float32)
    w3sum = W3.sum(axis=0).astype(np.float32)

    out = np.empty((N, 64), np.float32)
    for d in range(NCORES):
        oT = res.results[d]["outT"].astype(np.float32)  # [128, 3200]
        # col s*64 + j*16 + n_l, partition 64h+f <- node s*128 + (2j+h)*16 + n_l
        full = (oT.reshape(2, 64, N_SC, 4, 16).transpose(2, 3, 0, 4, 1)
                .reshape(N_PAD, 64))
        out[d * N_LOC:(d + 1) * N_LOC] = full[:N_LOC]
    out -= S_x[:, None] * w3sum[None, :]
    return np.nan_to_num(out, posinf=10000.0, neginf=-10000.0)
